# revision 10
# baseline (speedup 1.0000x reference)
"""Trainium2 Bass kernel for nn_AttentionSampling (sparse window attention block).

Sharding: 8 cores, data-parallel, 1024 windows (half a batch) per core; no
cross-core communication.

v2 design (vs v1 baseline at 174us):
- v-projection is linear, so the windowed weighted-sum runs FIRST on raw
  `value` (DVE, bf16) and only 1024 downsampled tokens get projected (4x less
  PE work on the v path). The w-weighted sum uses per-partition scalars
  (windows on partitions); the result is moved to the transposed domain with
  a DMA xbar transpose (no PE transposes, no PSUM->SBUF copies).
- Band scores extracted with tensor_tensor_reduce against the identity
  matrix (fused diagonal extract + fp8 descale), replacing mask-mult+reduce.
- fp8(e4m3) DoubleRow matmuls for q-proj, k-proj, scores and the FFN
  (weights pre-scaled x32, activations x8, descales folded into ACT scales /
  TTR scale / stt scalars). The v path (downsample + projection) stays bf16:
  fp8 there would put ~5% noise directly on the attention output.
- All elementwise DVE work in bf16 (2x mode); LN squares on the scalar
  engine (ACT Square); stats matmuls in bf16 with fp32 PSUM accumulation.
- 4 superblocks of 256 windows (2 attention blocks each) to shorten the
  serial LN/FFN tail; per-dt output DMA for the last superblock.
- Input DMAs batched (2 attention blocks per load, packed weight/bias
  tensors) and ordered so q-proj starts ~1.5us in.
"""

import sys
import types

try:
    import antenv.axon_hooks  # noqa: F401
except ImportError:
    _m = types.ModuleType("antenv.axon_hooks")
    _m.get_axon_ntff_profile_hook = lambda: None
    _m.set_axon_ntff_profile_hook = lambda h: None
    sys.modules["antenv.axon_hooks"] = _m
    try:
        import antenv

        antenv.axon_hooks = _m
    except ImportError:
        pass

import contextlib

import numpy as np

import concourse.bass as bass
import concourse.bacc as bacc_mod
import concourse.mybir as mybir
import concourse.tile as tile
from concourse.bass import ts, ds
from concourse.bass_utils import run_bass_kernel_spmd

FP32 = mybir.dt.float32
BF16 = mybir.dt.bfloat16
FP8 = mybir.dt.float8e4
AF = mybir.ActivationFunctionType
OP = mybir.AluOpType
DR = mybir.MatmulPerfMode.DoubleRow

B, SQ, SK, D, F = 4, 2048, 8192, 512, 4
NCORES = 8
WPC = B * SQ // NCORES        # 1024 windows (= tokens) per core
KPC = WPC * F                 # 4096 keys per core
NBLK = WPC // 128             # 8 attention blocks: 128 windows / 512 keys
NSB = 4                       # superblocks of 256 windows (2 blocks)
SBW = WPC // NSB              # 256
DT = D // 128                 # 4 d-tiles
EPS = 1e-5

# fp8 scale scheme: activations x8, weights x32 (host-side); descales folded.
AS, WS = 8.0, 32.0

# packed bias tensor rows (each a [128, DT] column tile)
BIAS_NAMES = ["bq8", "bk8", "b1_8", "b2", "g1", "gb1", "g2", "gb2", "g18", "gb18"]
BIX = {n: i for i, n in enumerate(BIAS_NAMES)}

_CACHE = {}


def build_program(bias_general: bool):
    nc = bacc_mod.Bacc(None, target_bir_lowering=False)

    qT_d = nc.dram_tensor("qT8", [D, WPC], FP8, kind="ExternalInput")
    kT_d = nc.dram_tensor("kT8", [D, KPC], FP8, kind="ExternalInput")
    vN_d = nc.dram_tensor("vN", [KPC, D], BF16, kind="ExternalInput")
    wqk_d = nc.dram_tensor("wqk8", [2 * D, D], FP8, kind="ExternalInput")
    w12_d = nc.dram_tensor("w12_8", [2 * D, D], FP8, kind="ExternalInput")
    wv_d = nc.dram_tensor("wv16", [D, D], BF16, kind="ExternalInput")
    bias_d = nc.dram_tensor("biaspack", [128, len(BIAS_NAMES), DT], FP32,
                            kind="ExternalInput")
    mask_d = nc.dram_tensor("cmask", [128, 512], FP32, kind="ExternalInput")
    ident_d = nc.dram_tensor("cident", [128, 128], FP32, kind="ExternalInput")
    bvrow_d = nc.dram_tensor("bvrow16", [D], BF16, kind="ExternalInput")
    outT_d = nc.dram_tensor("outT", [D, WPC], BF16, kind="ExternalOutput")

    qT_t = qT_d.rearrange("(o p) n -> p o n", p=128)
    kT_t = kT_d.rearrange("(o p) n -> p o n", p=128)
    vN_t = vN_d.rearrange("(j t w f) d -> j w t (f d)", j=NBLK // 2, t=2, w=128)
    wqk_t = wqk_d.rearrange("(o p) n -> p o n", p=128)   # [128, 8, 512]
    w12_t = w12_d.rearrange("(o p) n -> p o n", p=128)
    wv_t = wv_d.rearrange("(o p) n -> p o n", p=128)
    outT_t = outT_d.rearrange("(o p) n -> p o n", p=128)

    with tile.TileContext(nc) as tc, contextlib.ExitStack() as ctx:
        singles = ctx.enter_context(tc.tile_pool(name="singles", bufs=1))
        kin_p = ctx.enter_context(tc.tile_pool(name="kin", bufs=3))
        vin_p = ctx.enter_context(tc.tile_pool(name="vin", bufs=3))
        ktp_p = ctx.enter_context(tc.tile_pool(name="ktp", bufs=2))
        att_p = ctx.enter_context(tc.tile_pool(name="att", bufs=3))
        aot_p = ctx.enter_context(tc.tile_pool(name="aot", bufs=2))
        sb_p = ctx.enter_context(tc.tile_pool(name="sbp", bufs=2))
        small = ctx.enter_context(tc.tile_pool(name="small", bufs=3))
        ps_proj = ctx.enter_context(tc.tile_pool(name="ps_proj", bufs=2, space="PSUM"))
        ps_sc = ctx.enter_context(tc.tile_pool(name="ps_sc", bufs=2, space="PSUM"))
        ps_st = ctx.enter_context(tc.tile_pool(name="ps_st", bufs=1, space="PSUM"))
        ps_bc = ctx.enter_context(tc.tile_pool(name="ps_bc", bufs=1, space="PSUM"))

        # ---- constants; order matters: q-proj's deps first ----
        wqk = singles.tile([128, 2 * DT, 512], FP8, tag="wqk")
        nc.sync.dma_start(out=wqk, in_=wqk_t)
        biasp = singles.tile([128, len(BIAS_NAMES), DT], FP32, tag="biasp")
        nc.sync.dma_start(out=biasp, in_=bias_d[:, :, :])
        q_in = singles.tile([128, DT, WPC], FP8, tag="q_in")
        nc.sync.dma_start(out=q_in, in_=qT_t)
        mask = singles.tile([128, 512], FP32, tag="mask")
        nc.sync.dma_start(out=mask, in_=mask_d[:, :])
        identity = singles.tile([128, 128], FP32, tag="ident")
        nc.sync.dma_start(out=identity, in_=ident_d[:, :])
        wv_sb = singles.tile([128, DT, 512], BF16, tag="wv")
        nc.sync.dma_start(out=wv_sb, in_=wv_t)

        ones_col = singles.tile([128, 1], BF16, tag="ones_col")
        nc.gpsimd.memset(ones_col, 1.0)
        ones_row = singles.tile([1, 128], FP32, tag="ones_row")
        nc.gpsimd.memset(ones_row, 1.0)
        eps_t = singles.tile([1, 1], FP32, tag="eps")
        nc.gpsimd.memset(eps_t, EPS)
        if bias_general:
            bvrow = singles.tile([1, 512], BF16, tag="bvrow")
            nc.gpsimd.dma_start(
                out=bvrow, in_=bass.AP(tensor=bvrow_d, offset=0, ap=[[0, 1], [1, 512]])
            )

        def bias_ap(name, dt_=None):
            t = biasp[:, BIX[name], :]
            return t if dt_ is None else biasp[:, BIX[name], dt_ : dt_ + 1]

        late = {}

        def load_late():
            t = singles.tile([128, 2 * DT, 512], FP8, tag="w12")
            nc.sync.dma_start(out=t, in_=w12_t)
            late["w12"] = t

        qTp8 = singles.tile([128, DT, WPC], FP8, tag="qTp8")

        def proj_fp8(w_sb, w_off, bias_name, in_sb, in_off, n, out_cb):
            """out_cb(do, ps): ps = 256*(x @ W) for do-tile; x,W pre-scaled."""
            for do in range(DT):
                ps = ps_proj.tile([128, 512], FP32, tag="proj_ps", name="proj_ps")
                ps = ps[:, :n]
                for kk in range(0, DT, 2):
                    nc.tensor.matmul(
                        ps,
                        lhsT=w_sb[:, w_off + kk : w_off + kk + 2, ts(do, 128)],
                        rhs=in_sb[:, kk : kk + 2, ds(in_off, n)],
                        start=(kk == 0), stop=(kk == DT - 2), perf_mode=DR,
                    )
                out_cb(do, ps)

        # ---- phase 1: q projection (2 halves of 512 tokens) ----
        for h in range(2):
            def wq_out(do, ps, h=h):
                nc.scalar.activation(
                    out=qTp8[:, do, ds(h * 512, 512)], in_=ps, func=AF.Relu,
                    bias=bias_ap("bq8", do), scale=1.0 / WS,
                )
            proj_fp8(wqk, 0, "bq8", q_in, h * 512, 512, wq_out)

        # ---- attention blocks ----
        kv_tiles = [None, None]

        def emit_kv_load(j):  # loads blocks 2j, 2j+1
            k_in = kin_p.tile([128, DT, 1024], FP8, tag="k_in", name="k_in")
            nc.sync.dma_start(out=k_in, in_=kT_t[:, :, ts(j, 1024)])
            v_in = vin_p.tile([128, 2, 2048], BF16, tag="v_in", name="v_in")
            nc.sync.dma_start(out=v_in, in_=vN_t[j])
            kv_tiles[0], kv_tiles[1] = k_in, v_in

        def emit_block(b, aoT):
            t = b % 2
            if t == 0:
                emit_kv_load(b // 2)
            k_in, v_in = kv_tiles

            kTp8 = ktp_p.tile([128, DT, 512], FP8, tag="kTp8", name="kTp8")

            def wk_out(do, ps):
                nc.scalar.activation(
                    out=kTp8[:, do, :], in_=ps, func=AF.Relu,
                    bias=bias_ap("bk8", do), scale=1.0 / WS,
                )
            proj_fp8(wqk, DT, "bk8", k_in, t * 512, 512, wk_out)

            sc_ps = ps_sc.tile([128, 512], FP32, tag="sc_ps", name="sc_ps")
            for kk in range(0, DT, 2):
                nc.tensor.matmul(
                    sc_ps, lhsT=qTp8[:, kk : kk + 2, ts(b, 128)],
                    rhs=kTp8[:, kk : kk + 2, :],
                    start=(kk == 0), stop=(kk == DT - 2), perf_mode=DR,
                )
            # band extraction: wts[p, f] = sc[p, 4p+f] / AS^2 (descale in mask)
            sm = att_p.tile([128, 512], BF16, tag="sm", name="sm")
            nc.vector.tensor_tensor(sm, sc_ps, mask, op=OP.mult)
            wts = small.tile([128, F], FP32, tag="wts", name="wts")
            nc.vector.tensor_reduce(
                out=wts, in_=sm.rearrange("p (kw f) -> p f kw", f=F),
                axis=mybir.AxisListType.X, op=OP.add,
            )
            # windowed downsample of raw value (bf16)
            ao = att_p.tile([128, 512], BF16, tag="ao", name="ao")
            vv = v_in[:, t, :]
            nc.vector.tensor_scalar(
                out=ao, in0=vv[:, ts(0, 512)], scalar1=wts[:, 0:1], scalar2=None,
                op0=OP.mult,
            )
            for f in range(1, F):
                nc.vector.scalar_tensor_tensor(
                    out=ao, in0=vv[:, ts(f, 512)], scalar=wts[:, f : f + 1],
                    in1=ao, op0=OP.mult, op1=OP.add,
                )
            if bias_general:
                wsum = small.tile([128, 1], FP32, tag="wsum", name="wsum")
                nc.vector.tensor_reduce(
                    out=wsum, in_=wts, axis=mybir.AxisListType.X, op=OP.add
                )
                wsr_ps = ps_st.tile([1, 128], FP32, tag="wsr_ps", name="wsr_ps")
                nc.tensor.matmul(wsr_ps, lhsT=wsum, rhs=identity, start=True, stop=True)
                nc.scalar.activation(
                    out=aoT["wsrow"][:, ts(b % 2, 128)], in_=wsr_ps, func=AF.Copy
                )
            # move to transposed domain: aoT[p, o, w] = ao[w, 128o+p]
            nc.sync.dma_start_transpose(
                out=aoT["t"][:, :, ts(b % 2, 128)], in_=ao
            )

        def emit_aot(sb):
            t = aot_p.tile([128, DT, SBW], BF16, tag="aoT", name="aoT")
            r = {"t": t}
            if bias_general:
                r["wsrow"] = small.tile([1, SBW], BF16, tag="wsrow", name="wsrow")
            return r

        def emit_vproj_resid(sb, aoT):
            resid = sb_p.tile([128, DT, SBW], BF16, tag="resid", name="resid")
            for do in range(DT):
                ps = ps_proj.tile([128, 512], FP32, tag="proj_ps", name="vproj_ps")
                ps = ps[:, :SBW]
                for ki in range(DT):
                    nc.tensor.matmul(
                        ps, lhsT=wv_sb[:, ki, ts(do, 128)], rhs=aoT["t"][:, ki, :],
                        start=(ki == 0),
                        stop=(ki == DT - 1 and not bias_general),
                    )
                if bias_general:
                    nc.tensor.matmul(
                        ps, lhsT=bvrow[:, ts(do, 128)], rhs=aoT["wsrow"],
                        start=False, stop=True,
                    )
                nc.vector.scalar_tensor_tensor(
                    out=resid[:, do, :], in0=qTp8[:, do, ts(sb, SBW)],
                    scalar=1.0 / AS, in1=ps, op0=OP.mult, op1=OP.add,
                )
            return resid

        def emit_ln(x_sb, out_cb):
            """LayerNorm over D of x_sb [128, DT, SBW] (bf16, transposed).
            out_cb(dt, y2): consume normalized (pre-affine) tile."""
            sq = sb_p.tile([128, DT, SBW], BF16, tag="sq", name="sq")
            nc.scalar.activation(
                out=sq.rearrange("p a b -> p (a b)"),
                in_=x_sb.rearrange("p a b -> p (a b)"), func=AF.Square,
            )
            mean_ps = ps_st.tile([1, SBW], FP32, tag="st_mean", name="st_mean")
            for ki in range(DT):
                nc.tensor.matmul(
                    mean_ps, lhsT=ones_col, rhs=x_sb[:, ki, :],
                    start=(ki == 0), stop=(ki == DT - 1),
                )
            sq_ps = ps_st.tile([1, SBW], FP32, tag="st_sq", name="st_sq")
            for ki in range(DT):
                nc.tensor.matmul(
                    sq_ps, lhsT=ones_col, rhs=sq[:, ki, :],
                    start=(ki == 0), stop=(ki == DT - 1),
                )
            mean_sb = small.tile([1, SBW], FP32, tag="mean_sb", name="mean_sb")
            nc.scalar.activation(out=mean_sb, in_=mean_ps, func=AF.Copy, scale=1.0 / D)
            m2 = small.tile([1, SBW], FP32, tag="m2", name="m2")
            nc.scalar.activation(out=m2, in_=mean_ps, func=AF.Square, scale=1.0 / D)
            var = small.tile([1, SBW], FP32, tag="var", name="var")
            nc.vector.scalar_tensor_tensor(
                out=var, in0=sq_ps, scalar=1.0 / D, in1=m2,
                op0=OP.mult, op1=OP.subtract,
            )
            nc.scalar.activation(out=var, in_=var, func=AF.Sqrt, bias=eps_t, scale=1.0)
            rstd_sb = small.tile([1, SBW], FP32, tag="rstd_sb", name="rstd_sb")
            nc.vector.reciprocal_approx_fast(out=rstd_sb, in_=var)

            bc = {}
            for nm, row in (("mu", mean_sb), ("rs", rstd_sb)):
                bps = ps_bc.tile([128, SBW], FP32, tag="bc_ps", name="bc_" + nm)
                nc.tensor.matmul(bps, lhsT=ones_row, rhs=row, start=True, stop=True)
                bsb = small.tile([128, SBW], BF16, tag="bc_sb", name="bcs_" + nm)
                nc.scalar.activation(out=bsb, in_=bps, func=AF.Copy)
                bc[nm] = bsb
            for dt_ in range(DT):
                y = sq[:, dt_, :]  # reuse squares tile as scratch
                nc.vector.tensor_tensor(y, x_sb[:, dt_, :], bc["mu"], op=OP.subtract)
                nc.vector.tensor_tensor(y, y, bc["rs"], op=OP.mult)
                out_cb(dt_, y)

        def emit_ln1_ffn(sb, resid):
            xT = sb_p.tile([128, DT, SBW], BF16, tag="xT", name="xT")
            xT8 = sb_p.tile([128, DT, SBW], FP8, tag="xT8", name="xT8")

            def write_x(dt_, y):
                nc.scalar.activation(
                    out=xT[:, dt_, :], in_=y, func=AF.Identity,
                    bias=bias_ap("gb1", dt_), scale=bias_ap("g1", dt_),
                )
                nc.scalar.activation(
                    out=xT8[:, dt_, :], in_=y, func=AF.Identity,
                    bias=bias_ap("gb18", dt_), scale=bias_ap("g18", dt_),
                )
            emit_ln(resid, write_x)

            h8 = sb_p.tile([128, DT, SBW], FP8, tag="h8", name="h8")

            def w1_out(ht, ps):
                nc.scalar.activation(
                    out=h8[:, ht, :], in_=ps, func=AF.Relu,
                    bias=bias_ap("b1_8", ht), scale=1.0 / WS,
                )
            proj_fp8(late["w12"], 0, "b1_8", xT8, 0, SBW, w1_out)

            resid2 = sb_p.tile([128, DT, SBW], BF16, tag="resid2", name="resid2")

            def w2_out(do, ps):
                t2 = small.tile([128, SBW], BF16, tag="t2", name="t2")
                nc.scalar.activation(
                    out=t2, in_=ps, func=AF.Identity,
                    bias=bias_ap("b2", do), scale=1.0 / (AS * WS),
                )
                nc.vector.tensor_tensor(resid2[:, do, :], t2, xT[:, do, :], op=OP.add)
            proj_fp8(late["w12"], DT, "b2", h8, 0, SBW, w2_out)
            return resid2

        def emit_ln2_out(sb, resid2):
            out_sb = sb_p.tile([128, DT, SBW], BF16, tag="out_sb", name="out_sb")
            last = sb == NSB - 1

            def write_out(dt_, y):
                nc.scalar.activation(
                    out=out_sb[:, dt_, :], in_=y, func=AF.Identity,
                    bias=bias_ap("gb2", dt_), scale=bias_ap("g2", dt_),
                )
                if last:
                    nc.sync.dma_start(
                        out=outT_t[:, dt_, ts(sb, SBW)], in_=out_sb[:, dt_, :]
                    )
            emit_ln(resid2, write_out)
            if not last:
                nc.sync.dma_start(out=outT_t[:, :, ts(sb, SBW)], in_=out_sb)

        # ---- main schedule: interleave blocks with superblock mid/back ends ----
        aoTs = {}
        sbst = {}
        for sb in range(NSB):
            b0, b1 = 2 * sb, 2 * sb + 1
            aoTs[sb] = emit_aot(sb)
            emit_block(b0, aoTs[sb])
            if sb == 0:
                load_late()
            if sb >= 1:  # LN1+FFN of sb-1 overlaps this block pair's PE work
                sbst[sb - 1]["r2"] = emit_ln1_ffn(sb - 1, sbst[sb - 1]["resid"])
            emit_block(b1, aoTs[sb])
            if sb >= 1:
                emit_ln2_out(sb - 1, sbst[sb - 1]["r2"])
            sbst[sb] = {"resid": emit_vproj_resid(sb, aoTs[sb])}
        r2 = emit_ln1_ffn(NSB - 1, sbst[NSB - 1]["resid"])
        emit_ln2_out(NSB - 1, r2)

    nc.finalize()
    return nc


def _band_mask():
    p = np.arange(128)[:, None]
    k = np.arange(512)[None, :]
    band = (k - 4 * p >= 0) & (k - 4 * p <= 3)
    return band.astype(np.float32) / (AS * AS)


def _host_prep(inputs):
    """Shared (per-core-invariant) tensors, host-side precompute."""
    import ml_dtypes

    E4 = ml_dtypes.float8_e4m3fn
    BF = ml_dtypes.bfloat16
    f32 = lambda x: np.asarray(x, dtype=np.float32)

    def colpack(v):  # [D] -> [128, DT] column tile layout (d = o*128 + p)
        return f32(v).reshape(DT, 128).T

    wq, wk, wv = f32(inputs["w_q"]), f32(inputs["w_k"]), f32(inputs["w_v"])
    w1, w2 = f32(inputs["ffn_w1"]), f32(inputs["ffn_w2"])
    g1, gb1 = f32(inputs["ln1_g"]), f32(inputs["ln1_b"])

    shared = {
        "wqk8": np.ascontiguousarray(
            np.concatenate([wq, wk], axis=0) * WS).astype(E4),
        "w12_8": np.ascontiguousarray(
            np.concatenate([w1, w2], axis=0) * WS).astype(E4),
        "wv16": np.ascontiguousarray(wv).astype(BF),
        "cident": np.eye(128, dtype=np.float32),
        "cmask": _band_mask(),
        "bvrow16": f32(inputs["b_v"]).astype(BF),
    }
    bias_cols = {
        "bq8": AS * f32(inputs["b_q"]),
        "bk8": AS * f32(inputs["b_k"]),
        "b1_8": AS * f32(inputs["ffn_b1"]),
        "b2": f32(inputs["ffn_b2"]),
        "g1": g1, "gb1": gb1,
        "g2": f32(inputs["ln2_g"]), "gb2": f32(inputs["ln2_b"]),
        "g18": AS * g1, "gb18": AS * gb1,
    }
    bp = np.stack([colpack(bias_cols[n]) for n in BIAS_NAMES], axis=1)
    shared["biaspack"] = np.ascontiguousarray(bp)  # [128, NB, DT]
    return shared


def kernel(**inputs):
    import ml_dtypes

    E4 = ml_dtypes.float8_e4m3fn
    BF = ml_dtypes.bfloat16

    bias_general = bool(np.any(np.asarray(inputs["b_v"], dtype=np.float32)))
    key_ = ("prog", bias_general)
    if key_ not in _CACHE:
        _CACHE[key_] = build_program(bias_general)
    nc = _CACHE[key_]

    shared = _host_prep(inputs)
    query = np.asarray(inputs["query"], dtype=np.float32)
    key_t = np.asarray(inputs["key"], dtype=np.float32)
    value = np.asarray(inputs["value"], dtype=np.float32)

    in_maps = []
    for c in range(NCORES):
        bi, half = c // 2, c % 2
        w0 = half * WPC
        m = dict(shared)
        m["qT8"] = np.ascontiguousarray(query[bi, w0 : w0 + WPC, :].T * AS).astype(E4)
        m["kT8"] = np.ascontiguousarray(
            key_t[bi, w0 * F : (w0 + WPC) * F, :].T * AS).astype(E4)
        m["vN"] = np.ascontiguousarray(
            value[bi, w0 * F : (w0 + WPC) * F, :]).astype(BF)
        in_maps.append(m)

    res = run_bass_kernel_spmd(nc, in_maps, core_ids=list(range(NCORES)))
    _CACHE["last_result"] = res
    out = np.empty((B, SQ, D), dtype=np.float32)
    for c in range(NCORES):
        bi, half = c // 2, c % 2
        w0 = half * WPC
        out[bi, w0 : w0 + WPC, :] = res.results[c]["outT"].astype(np.float32).T
    return out


# revision 13
# speedup vs baseline: 1.0606x; 1.0606x over previous
"""Trainium2 Bass kernel for nn_AttentionSampling (sparse window attention block).

Sharding: 8 cores, data-parallel, 1024 windows (half a batch) per core; no
cross-core communication.

v3 design (baseline was 174us; v2's fp8/DoubleRow was a wash on PE time and
cost error margin, so this is all-bf16):
- v-projection is linear, so the windowed weighted-sum runs FIRST on raw
  `value` (DVE, bf16, windows on partitions) and only 1024 downsampled
  tokens get projected (4x less PE work on the v path). The result moves to
  the transposed domain with DMA xbar transposes (no PE transposes).
- ReLU epilogues split between the scalar and vector engines per d-tile to
  balance the two (ACT alone was the v2 bottleneck at 99us busy).
- LN1's affine is folded into the FFN weights host-side (W1' = diag(g1) W1,
  b1' = gb1 @ W1 + b1) and into the residual-2 accumulate (per-partition
  scalar multiply), so the LN1 output is never materialized when ln1_b,
  ffn_b2 and b_v are zero (the common case; a general fallback program
  handles nonzero ones).
- LN squares on DVE (bf16 2x), stats matmuls bf16 with fp32 PSUM
  accumulation, mean/rstd rows bf16 for cheap broadcast matmuls.
- 4 superblocks of 256 windows; per-dt output DMA on the last superblock;
  batched input DMAs (2 attention blocks per load, packed weights/biases).
"""

import sys
import types

try:
    import antenv.axon_hooks  # noqa: F401
except ImportError:
    _m = types.ModuleType("antenv.axon_hooks")
    _m.get_axon_ntff_profile_hook = lambda: None
    _m.set_axon_ntff_profile_hook = lambda h: None
    sys.modules["antenv.axon_hooks"] = _m
    try:
        import antenv

        antenv.axon_hooks = _m
    except ImportError:
        pass

import contextlib

import numpy as np

import concourse.bass as bass
import concourse.bacc as bacc_mod
import concourse.mybir as mybir
import concourse.tile as tile
from concourse.bass import ts, ds
from concourse.bass_utils import run_bass_kernel_spmd

FP32 = mybir.dt.float32
BF16 = mybir.dt.bfloat16
AF = mybir.ActivationFunctionType
OP = mybir.AluOpType

B, SQ, SK, D, F = 4, 2048, 8192, 512, 4
NCORES = 8
WPC = B * SQ // NCORES        # 1024 windows (= tokens) per core
KPC = WPC * F                 # 4096 keys per core
NBLK = WPC // 128             # 8 attention blocks: 128 windows / 512 keys
NSB = 4                       # superblocks of 256 windows (2 blocks)
SBW = WPC // NSB              # 256
DT = D // 128                 # 4 d-tiles
EPS = 1e-5

BIAS_NAMES = ["bq", "bk", "b1", "b2", "g1", "gb1", "g2", "gb2"]
BIX = {n: i for i, n in enumerate(BIAS_NAMES)}

_CACHE = {}


def build_program(general: bool):
    nc = bacc_mod.Bacc(None, target_bir_lowering=False)

    qT_d = nc.dram_tensor("qT16", [D, WPC], BF16, kind="ExternalInput")
    kT_d = nc.dram_tensor("kT16", [D, KPC], BF16, kind="ExternalInput")
    vN_d = nc.dram_tensor("vN", [KPC, D], BF16, kind="ExternalInput")
    wqk_d = nc.dram_tensor("wqk16", [2 * D, D], BF16, kind="ExternalInput")
    w12_d = nc.dram_tensor("w12_16", [2 * D, D], BF16, kind="ExternalInput")
    wv_d = nc.dram_tensor("wv16", [D, D], BF16, kind="ExternalInput")
    bias_d = nc.dram_tensor("biaspack", [128, len(BIAS_NAMES), DT], FP32,
                            kind="ExternalInput")
    mask_d = nc.dram_tensor("cmask", [128, 512], FP32, kind="ExternalInput")
    ident_d = nc.dram_tensor("cident", [128, 128], FP32, kind="ExternalInput")
    bvrow_d = nc.dram_tensor("bvrow16", [D], BF16, kind="ExternalInput")
    outT_d = nc.dram_tensor("outT", [D, WPC], BF16, kind="ExternalOutput")

    qT_t = qT_d.rearrange("(o p) n -> p o n", p=128)
    kT_t = kT_d.rearrange("(o p) n -> p o n", p=128)
    vN_t = vN_d.rearrange("(j t w f) d -> j w t (f d)", j=NBLK // 2, t=2, w=128)
    wqk_t = wqk_d.rearrange("(o p) n -> p o n", p=128)   # [128, 8, 512]
    w12_t = w12_d.rearrange("(o p) n -> p o n", p=128)
    wv_t = wv_d.rearrange("(o p) n -> p o n", p=128)
    outT_t = outT_d.rearrange("(o p) n -> p o n", p=128)

    with tile.TileContext(nc) as tc, contextlib.ExitStack() as ctx:
        singles = ctx.enter_context(tc.tile_pool(name="singles", bufs=1))
        kin_p = ctx.enter_context(tc.tile_pool(name="kin", bufs=3))
        vin_p = ctx.enter_context(tc.tile_pool(name="vin", bufs=3))
        ktp_p = ctx.enter_context(tc.tile_pool(name="ktp", bufs=2))
        att_p = ctx.enter_context(tc.tile_pool(name="att", bufs=3))
        aot_p = ctx.enter_context(tc.tile_pool(name="aot", bufs=2))
        sb_p = ctx.enter_context(tc.tile_pool(name="sbp", bufs=2))
        small = ctx.enter_context(tc.tile_pool(name="small", bufs=3))
        ps_proj = ctx.enter_context(tc.tile_pool(name="ps_proj", bufs=3, space="PSUM"))
        ps_sc = ctx.enter_context(tc.tile_pool(name="ps_sc", bufs=2, space="PSUM"))
        ps_st = ctx.enter_context(tc.tile_pool(name="ps_st", bufs=1, space="PSUM"))
        ps_bc = ctx.enter_context(tc.tile_pool(name="ps_bc", bufs=1, space="PSUM"))

        # ---- constants; order matters: q-proj's deps first ----
        wqk = singles.tile([128, 2 * DT, 512], BF16, tag="wqk")
        nc.sync.dma_start(out=wqk, in_=wqk_t)
        biasp = singles.tile([128, len(BIAS_NAMES), DT], FP32, tag="biasp")
        nc.sync.dma_start(out=biasp, in_=bias_d[:, :, :])
        q_in = singles.tile([128, DT, WPC], BF16, tag="q_in")
        nc.sync.dma_start(out=q_in, in_=qT_t)
        mask = singles.tile([128, 512], FP32, tag="mask")
        nc.sync.dma_start(out=mask, in_=mask_d[:, :])
        wv_sb = singles.tile([128, DT, 512], BF16, tag="wv")
        nc.sync.dma_start(out=wv_sb, in_=wv_t)
        if general:
            identity = singles.tile([128, 128], FP32, tag="ident")
            nc.sync.dma_start(out=identity, in_=ident_d[:, :])
            bvrow = singles.tile([1, 512], BF16, tag="bvrow")
            nc.gpsimd.dma_start(
                out=bvrow, in_=bass.AP(tensor=bvrow_d, offset=0, ap=[[0, 1], [1, 512]])
            )

        ones_col = singles.tile([128, 1], BF16, tag="ones_col")
        nc.gpsimd.memset(ones_col, 1.0)
        ones_row = singles.tile([1, 128], FP32, tag="ones_row")
        nc.gpsimd.memset(ones_row, 1.0)
        eps_t = singles.tile([1, 1], FP32, tag="eps")
        nc.gpsimd.memset(eps_t, EPS)

        def bias_ap(name, dt_):
            return biasp[:, BIX[name], dt_ : dt_ + 1]

        late = {}

        def load_late():
            t = singles.tile([128, 2 * DT, 512], BF16, tag="w12")
            nc.sync.dma_start(out=t, in_=w12_t)
            late["w12"] = t

        qTp = singles.tile([128, DT, WPC], BF16, tag="qTp")

        def proj(w_sb, w_off, bias_name, in_sb, in_off, n, out_sb_ap):
            """out = relu(x @ W + b); epilogue split ACT (do 0,2) / DVE (1,3)."""
            for do in range(DT):
                ps = ps_proj.tile([128, 512], FP32, tag="proj_ps", name="proj_ps")
                ps = ps[:, :n]
                for ki in range(DT):
                    nc.tensor.matmul(
                        ps, lhsT=w_sb[:, w_off + ki, ts(do, 128)],
                        rhs=in_sb[:, ki, ds(in_off, n)],
                        start=(ki == 0), stop=(ki == DT - 1),
                    )
                if do % 2 == 0:
                    nc.scalar.activation(
                        out=out_sb_ap(do), in_=ps, func=AF.Relu,
                        bias=bias_ap(bias_name, do), scale=1.0,
                    )
                else:
                    nc.vector.tensor_scalar(
                        out=out_sb_ap(do), in0=ps, scalar1=bias_ap(bias_name, do),
                        scalar2=0.0, op0=OP.add, op1=OP.max,
                    )

        # ---- phase 1: q projection (2 halves of 512 tokens) ----
        for h in range(2):
            proj(wqk, 0, "bq", q_in, h * 512, 512,
                 lambda do, h=h: qTp[:, do, ds(h * 512, 512)])

        # ---- attention blocks ----
        kv_tiles = [None, None]

        def emit_kv_load(j):  # loads blocks 2j, 2j+1
            k_in = kin_p.tile([128, DT, 1024], BF16, tag="k_in", name="k_in")
            nc.sync.dma_start(out=k_in, in_=kT_t[:, :, ts(j, 1024)])
            v_in = vin_p.tile([128, 2, 2048], BF16, tag="v_in", name="v_in")
            nc.sync.dma_start(out=v_in, in_=vN_t[j])
            kv_tiles[0], kv_tiles[1] = k_in, v_in

        def emit_block(b, aoT):
            t = b % 2
            if t == 0:
                emit_kv_load(b // 2)
            k_in, v_in = kv_tiles

            kTp = ktp_p.tile([128, DT, 512], BF16, tag="kTp", name="kTp")
            proj(wqk, DT, "bk", k_in, t * 512, 512, lambda do: kTp[:, do, :])

            sc_ps = ps_sc.tile([128, 512], FP32, tag="sc_ps", name="sc_ps")
            for ki in range(DT):
                nc.tensor.matmul(
                    sc_ps, lhsT=qTp[:, ki, ts(b, 128)], rhs=kTp[:, ki, :],
                    start=(ki == 0), stop=(ki == DT - 1),
                )
            # band extraction: wts[p, f] = sc[p, 4p+f]
            sm = att_p.tile([128, 512], BF16, tag="sm", name="sm")
            nc.vector.tensor_tensor(sm, sc_ps, mask, op=OP.mult)
            wts = small.tile([128, F], FP32, tag="wts", name="wts")
            nc.vector.tensor_reduce(
                out=wts, in_=sm.rearrange("p (kw f) -> p f kw", f=F),
                axis=mybir.AxisListType.X, op=OP.add,
            )
            # windowed downsample of raw value (bf16)
            ao = att_p.tile([128, 512], BF16, tag="ao", name="ao")
            vv = v_in[:, t, :]
            nc.vector.tensor_scalar(
                out=ao, in0=vv[:, ts(0, 512)], scalar1=wts[:, 0:1], scalar2=None,
                op0=OP.mult,
            )
            for f in range(1, F):
                nc.vector.scalar_tensor_tensor(
                    out=ao, in0=vv[:, ts(f, 512)], scalar=wts[:, f : f + 1],
                    in1=ao, op0=OP.mult, op1=OP.add,
                )
            if general:
                wsum = small.tile([128, 1], FP32, tag="wsum", name="wsum")
                nc.vector.tensor_reduce(
                    out=wsum, in_=wts, axis=mybir.AxisListType.X, op=OP.add
                )
                wsr_ps = ps_st.tile([1, 128], FP32, tag="wsr_ps", name="wsr_ps")
                nc.tensor.matmul(wsr_ps, lhsT=wsum, rhs=identity, start=True, stop=True)
                nc.scalar.activation(
                    out=aoT["wsrow"][:, ts(b % 2, 128)], in_=wsr_ps, func=AF.Copy
                )
            # move to transposed domain: aoT[p, o, w] = ao[w, 128o+p]
            nc.sync.dma_start_transpose(
                out=aoT["t"][:, :, ts(b % 2, 128)], in_=ao
            )

        def emit_aot(sb):
            t = aot_p.tile([128, DT, SBW], BF16, tag="aoT", name="aoT")
            r = {"t": t}
            if general:
                r["wsrow"] = small.tile([1, SBW], BF16, tag="wsrow", name="wsrow")
            return r

        def emit_vproj_resid(sb, aoT):
            resid = sb_p.tile([128, DT, SBW], BF16, tag="resid", name="resid")
            for do in range(DT):
                ps = ps_proj.tile([128, 512], FP32, tag="proj_ps", name="vproj_ps")
                ps = ps[:, :SBW]
                for ki in range(DT):
                    nc.tensor.matmul(
                        ps, lhsT=wv_sb[:, ki, ts(do, 128)], rhs=aoT["t"][:, ki, :],
                        start=(ki == 0),
                        stop=(ki == DT - 1 and not general),
                    )
                if general:
                    nc.tensor.matmul(
                        ps, lhsT=bvrow[:, ts(do, 128)], rhs=aoT["wsrow"],
                        start=False, stop=True,
                    )
                nc.vector.tensor_tensor(
                    resid[:, do, :], ps, qTp[:, do, ts(sb, SBW)], op=OP.add
                )
            return resid

        def emit_ln(x_sb, out_cb):
            """LayerNorm over D of x_sb [128, DT, SBW] (bf16, transposed).
            out_cb(dt, y2): consume normalized (pre-affine) tile; y2 tiles
            stay alive in the returned scratch tile."""
            scr = sb_p.tile([128, DT, SBW], BF16, tag="scr", name="scr")
            nc.vector.tensor_tensor(
                scr.rearrange("p a b -> p (a b)"),
                x_sb.rearrange("p a b -> p (a b)"),
                x_sb.rearrange("p a b -> p (a b)"), op=OP.mult,
            )
            mean_ps = ps_st.tile([1, SBW], FP32, tag="st_mean", name="st_mean")
            for ki in range(DT):
                nc.tensor.matmul(
                    mean_ps, lhsT=ones_col, rhs=x_sb[:, ki, :],
                    start=(ki == 0), stop=(ki == DT - 1),
                )
            sq_ps = ps_st.tile([1, SBW], FP32, tag="st_sq", name="st_sq")
            for ki in range(DT):
                nc.tensor.matmul(
                    sq_ps, lhsT=ones_col, rhs=scr[:, ki, :],
                    start=(ki == 0), stop=(ki == DT - 1),
                )
            mean_sb = small.tile([1, SBW], FP32, tag="mean_sb", name="mean_sb")
            nc.scalar.activation(out=mean_sb, in_=mean_ps, func=AF.Copy, scale=1.0 / D)
            m2 = small.tile([1, SBW], FP32, tag="m2", name="m2")
            nc.scalar.activation(out=m2, in_=mean_ps, func=AF.Square, scale=1.0 / D)
            var = small.tile([1, SBW], FP32, tag="var", name="var")
            nc.vector.scalar_tensor_tensor(
                out=var, in0=sq_ps, scalar=1.0 / D, in1=m2,
                op0=OP.mult, op1=OP.subtract,
            )
            nc.scalar.activation(out=var, in_=var, func=AF.Sqrt, bias=eps_t, scale=1.0)
            rstd_sb = small.tile([1, SBW], FP32, tag="rstd_sb", name="rstd_sb")
            nc.vector.reciprocal_approx_fast(out=rstd_sb, in_=var)

            bc = {}
            for nm, row in (("mu", mean_sb), ("rs", rstd_sb)):
                bps = ps_bc.tile([128, SBW], FP32, tag="bc_ps", name="bc_" + nm)
                nc.tensor.matmul(bps, lhsT=ones_row, rhs=row, start=True, stop=True)
                bsb = small.tile([128, SBW], BF16, tag="bc_sb", name="bcs_" + nm)
                nc.scalar.activation(out=bsb, in_=bps, func=AF.Copy)
                bc[nm] = bsb
            for dt_ in range(DT):
                y = scr[:, dt_, :]  # reuse squares tile as y2 output
                nc.vector.tensor_tensor(y, x_sb[:, dt_, :], bc["mu"], op=OP.subtract)
                nc.vector.tensor_tensor(y, y, bc["rs"], op=OP.mult)
                out_cb(dt_, y)
            return scr

        def emit_ln1_ffn(sb, resid):
            # fast path: ffn1 consumes y2 directly (g1 folded into W1 host-side,
            # gb1 = 0); resid2 = y2*g1 + ffn2_psum via stt (b2 = 0).
            if general:
                xT = sb_p.tile([128, DT, SBW], BF16, tag="xT", name="xT")

                def write_x(dt_, y):
                    nc.scalar.activation(
                        out=xT[:, dt_, :], in_=y, func=AF.Identity,
                        bias=bias_ap("gb1", dt_), scale=bias_ap("g1", dt_),
                    )
                y2 = emit_ln(resid, write_x)
                ffn_in = xT
            else:
                y2 = emit_ln(resid, lambda dt_, y: None)
                ffn_in = y2

            hT = sb_p.tile([128, DT, SBW], BF16, tag="hT", name="hT")
            proj(late["w12"], 0, "b1", ffn_in, 0, SBW, lambda ht: hT[:, ht, :])

            resid2 = sb_p.tile([128, DT, SBW], BF16, tag="resid2", name="resid2")
            for do in range(DT):
                ps = ps_proj.tile([128, 512], FP32, tag="proj_ps", name="ffn2_ps")
                ps = ps[:, :SBW]
                for ki in range(DT):
                    nc.tensor.matmul(
                        ps, lhsT=late["w12"][:, DT + ki, ts(do, 128)],
                        rhs=hT[:, ki, :],
                        start=(ki == 0), stop=(ki == DT - 1),
                    )
                if general:
                    nc.vector.scalar_tensor_tensor(
                        out=resid2[:, do, :], in0=ps, scalar=bias_ap("b2", do),
                        in1=ffn_in[:, do, :], op0=OP.add, op1=OP.add,
                    )
                else:
                    nc.vector.scalar_tensor_tensor(
                        out=resid2[:, do, :], in0=y2[:, do, :],
                        scalar=bias_ap("g1", do), in1=ps, op0=OP.mult, op1=OP.add,
                    )
            return resid2

        def emit_ln2_out(sb, resid2):
            out_sb = sb_p.tile([128, DT, SBW], BF16, tag="out_sb", name="out_sb")
            last = sb == NSB - 1

            def write_out(dt_, y):
                nc.scalar.activation(
                    out=out_sb[:, dt_, :], in_=y, func=AF.Identity,
                    bias=bias_ap("gb2", dt_), scale=bias_ap("g2", dt_),
                )
                if last:
                    nc.sync.dma_start(
                        out=outT_t[:, dt_, ts(sb, SBW)], in_=out_sb[:, dt_, :]
                    )
            emit_ln(resid2, write_out)
            if not last:
                nc.sync.dma_start(out=outT_t[:, :, ts(sb, SBW)], in_=out_sb)

        # ---- main schedule: interleave blocks with superblock mid/back ends ----
        aoTs = {}
        sbst = {}
        for sb in range(NSB):
            aoTs[sb] = emit_aot(sb)
            emit_block(2 * sb, aoTs[sb])
            if sb == 0:
                load_late()
            if sb >= 1:  # LN1+FFN of sb-1 overlaps this block pair's PE work
                sbst[sb - 1]["r2"] = emit_ln1_ffn(sb - 1, sbst[sb - 1]["resid"])
            emit_block(2 * sb + 1, aoTs[sb])
            if sb >= 1:
                emit_ln2_out(sb - 1, sbst[sb - 1]["r2"])
            sbst[sb] = {"resid": emit_vproj_resid(sb, aoTs[sb])}
        r2 = emit_ln1_ffn(NSB - 1, sbst[NSB - 1]["resid"])
        emit_ln2_out(NSB - 1, r2)

    nc.finalize()
    return nc


def _band_mask():
    p = np.arange(128)[:, None]
    k = np.arange(512)[None, :]
    band = (k - 4 * p >= 0) & (k - 4 * p <= 3)
    return band.astype(np.float32)


def _is_general(inputs):
    f32 = lambda n: np.asarray(inputs[n], dtype=np.float32)
    return bool(
        np.any(f32("b_v")) or np.any(f32("ln1_b")) or np.any(f32("ffn_b2"))
    )


def _host_prep(inputs, general):
    """Shared (per-core-invariant) tensors, host-side precompute."""
    import ml_dtypes

    BF = ml_dtypes.bfloat16
    f32 = lambda x: np.asarray(x, dtype=np.float32)

    def colpack(v):  # [D] -> [128, DT] column tile layout (d = o*128 + p)
        return f32(v).reshape(DT, 128).T

    wq, wk, wv = f32(inputs["w_q"]), f32(inputs["w_k"]), f32(inputs["w_v"])
    w1, w2 = f32(inputs["ffn_w1"]), f32(inputs["ffn_w2"])
    g1 = f32(inputs["ln1_g"])
    if not general:
        w1 = w1 * g1[:, None]  # fold LN1 gain into FFN1 (gb1 == 0)

    shared = {
        "wqk16": np.ascontiguousarray(np.concatenate([wq, wk], axis=0)).astype(BF),
        "w12_16": np.ascontiguousarray(np.concatenate([w1, w2], axis=0)).astype(BF),
        "wv16": np.ascontiguousarray(wv).astype(BF),
        "cident": np.eye(128, dtype=np.float32),
        "cmask": _band_mask(),
        "bvrow16": f32(inputs["b_v"]).astype(BF),
    }
    bias_cols = {
        "bq": f32(inputs["b_q"]), "bk": f32(inputs["b_k"]),
        "b1": f32(inputs["ffn_b1"]), "b2": f32(inputs["ffn_b2"]),
        "g1": g1, "gb1": f32(inputs["ln1_b"]),
        "g2": f32(inputs["ln2_g"]), "gb2": f32(inputs["ln2_b"]),
    }
    bp = np.stack([colpack(bias_cols[n]) for n in BIAS_NAMES], axis=1)
    shared["biaspack"] = np.ascontiguousarray(bp)  # [128, NB, DT]
    return shared


def kernel(**inputs):
    import ml_dtypes

    BF = ml_dtypes.bfloat16

    general = _is_general(inputs)
    key_ = ("prog", general)
    if key_ not in _CACHE:
        _CACHE[key_] = build_program(general)
    nc = _CACHE[key_]

    shared = _host_prep(inputs, general)
    query = np.asarray(inputs["query"], dtype=np.float32)
    key_t = np.asarray(inputs["key"], dtype=np.float32)
    value = np.asarray(inputs["value"], dtype=np.float32)

    in_maps = []
    for c in range(NCORES):
        bi, half = c // 2, c % 2
        w0 = half * WPC
        m = dict(shared)
        m["qT16"] = np.ascontiguousarray(query[bi, w0 : w0 + WPC, :].T).astype(BF)
        m["kT16"] = np.ascontiguousarray(
            key_t[bi, w0 * F : (w0 + WPC) * F, :].T).astype(BF)
        m["vN"] = np.ascontiguousarray(
            value[bi, w0 * F : (w0 + WPC) * F, :]).astype(BF)
        in_maps.append(m)

    res = run_bass_kernel_spmd(nc, in_maps, core_ids=list(range(NCORES)))
    _CACHE["last_result"] = res
    out = np.empty((B, SQ, D), dtype=np.float32)
    for c in range(NCORES):
        bi, half = c // 2, c % 2
        w0 = half * WPC
        out[bi, w0 : w0 + WPC, :] = res.results[c]["outT"].astype(np.float32).T
    return out


# revision 16
# speedup vs baseline: 1.2509x; 1.1794x over previous
"""Trainium2 Bass kernel for nn_AttentionSampling (sparse window attention block).

Sharding: 8 cores, data-parallel, 1024 windows (half a batch) per core; no
cross-core communication. All-bf16 matmuls (fp8/DoubleRow measured as a wash
on PE issue rate while costing error margin).

v4 design (vs 174us v1 baseline):
- v-projection is linear, so the windowed weighted-sum runs FIRST on raw
  `value` (DVE, windows on partitions) and only the 1024 downsampled tokens
  get projected (4x less PE work on the v path); the result moves to the
  transposed domain with DMA xbar transposes (no PE transposes, no
  PSUM->SBUF copies).
- LN1's affine folds into the FFN weights host-side (W1' = diag(g1) W1) and
  into the residual-2 accumulate (per-partition scalar), so the LN1 output
  is never materialized when ln1_b/ffn_b2/b_v are zero (a general program
  handles nonzero ones).
- Token range processed as chunks [512, 256, 256]: the big chunk keeps
  matmuls at N=512 (LDWEIGHTS fully hidden, 216ns/MM), the small tail
  chunks pipeline the serial LN/FFN dependency chain against remaining
  attention blocks.
- LN squares + applies on DVE (bf16 2x), stats matmuls bf16 with fp32 PSUM
  accumulation; relu epilogues on ACT; v loads and output stores on the
  GpSimd software DGE queue so the sync ring only carries q/k/weights and
  the xbar transposes.
"""

import sys
import types

try:
    import antenv.axon_hooks  # noqa: F401
except ImportError:
    _m = types.ModuleType("antenv.axon_hooks")
    _m.get_axon_ntff_profile_hook = lambda: None
    _m.set_axon_ntff_profile_hook = lambda h: None
    sys.modules["antenv.axon_hooks"] = _m
    try:
        import antenv

        antenv.axon_hooks = _m
    except ImportError:
        pass

import contextlib

import numpy as np

import concourse.bass as bass
import concourse.bacc as bacc_mod
import concourse.mybir as mybir
import concourse.tile as tile
from concourse.bass import ts, ds
from concourse.bass_utils import run_bass_kernel_spmd

FP32 = mybir.dt.float32
BF16 = mybir.dt.bfloat16
AF = mybir.ActivationFunctionType
OP = mybir.AluOpType

B, SQ, SK, D, F = 4, 2048, 8192, 512, 4
NCORES = 8
WPC = B * SQ // NCORES        # 1024 windows (= tokens) per core
KPC = WPC * F                 # 4096 keys per core
NBLK = WPC // 128             # 8 attention blocks: 128 windows / 512 keys
DT = D // 128                 # 4 d-tiles
EPS = 1e-5

# (col0, width, [blocks]) chunks of the token range
CHUNKS = [(0, 512, (0, 1, 2, 3)), (512, 256, (4, 5)), (768, 256, (6, 7))]

BIAS_NAMES = ["bq", "bk", "b1", "b2", "g1", "gb1", "g2", "gb2"]
BIX = {n: i for i, n in enumerate(BIAS_NAMES)}

_CACHE = {}


def build_program(general: bool):
    nc = bacc_mod.Bacc(None, target_bir_lowering=False)

    qT_d = nc.dram_tensor("qT16", [D, WPC], BF16, kind="ExternalInput")
    kT_d = nc.dram_tensor("kT16", [D, KPC], BF16, kind="ExternalInput")
    vN_d = nc.dram_tensor("vN", [KPC, D], BF16, kind="ExternalInput")
    wqk_d = nc.dram_tensor("wqk16", [2 * D, D], BF16, kind="ExternalInput")
    w12_d = nc.dram_tensor("w12_16", [2 * D, D], BF16, kind="ExternalInput")
    wv_d = nc.dram_tensor("wv16", [D, D], BF16, kind="ExternalInput")
    bias_d = nc.dram_tensor("biaspack", [128, len(BIAS_NAMES), DT], FP32,
                            kind="ExternalInput")
    mask_d = nc.dram_tensor("cmask", [128, 512], FP32, kind="ExternalInput")
    ident_d = nc.dram_tensor("cident", [128, 128], FP32, kind="ExternalInput")
    bvrow_d = nc.dram_tensor("bvrow16", [D], BF16, kind="ExternalInput")
    outT_d = nc.dram_tensor("outT", [D, WPC], BF16, kind="ExternalOutput")

    qT_t = qT_d.rearrange("(o p) n -> p o n", p=128)
    kT_t = kT_d.rearrange("(o p) n -> p o n", p=128)
    vN_t = vN_d.rearrange("(j t w f) d -> j w t (f d)", j=NBLK // 2, t=2, w=128)
    wqk_t = wqk_d.rearrange("(o p) n -> p o n", p=128)   # [128, 8, 512]
    w12_t = w12_d.rearrange("(o p) n -> p o n", p=128)
    wv_t = wv_d.rearrange("(o p) n -> p o n", p=128)
    outT_t = outT_d.rearrange("(o p) n -> p o n", p=128)

    with tile.TileContext(nc) as tc, contextlib.ExitStack() as ctx:
        singles = ctx.enter_context(tc.tile_pool(name="singles", bufs=1))
        kin_p = ctx.enter_context(tc.tile_pool(name="kin", bufs=3))
        vin_p = ctx.enter_context(tc.tile_pool(name="vin", bufs=3))
        ktp_p = ctx.enter_context(tc.tile_pool(name="ktp", bufs=2))
        att_p = ctx.enter_context(tc.tile_pool(name="att", bufs=3))
        aot_p = ctx.enter_context(tc.tile_pool(name="aot", bufs=2))
        sb_p = ctx.enter_context(tc.tile_pool(name="sbp", bufs=2))
        small = ctx.enter_context(tc.tile_pool(name="small", bufs=3))
        ps_proj = ctx.enter_context(tc.tile_pool(name="ps_proj", bufs=3, space="PSUM"))
        ps_sc = ctx.enter_context(tc.tile_pool(name="ps_sc", bufs=2, space="PSUM"))
        ps_st = ctx.enter_context(tc.tile_pool(name="ps_st", bufs=1, space="PSUM"))
        ps_bc = ctx.enter_context(tc.tile_pool(name="ps_bc", bufs=1, space="PSUM"))

        # ---- constants; ring order matters: q-proj's deps first, then k0 ----
        wqk = singles.tile([128, 2 * DT, 512], BF16, tag="wqk")
        nc.sync.dma_start(out=wqk[:, :DT, :], in_=wqk_t[:, :DT, :])      # wq
        q_in = singles.tile([128, DT, WPC], BF16, tag="q_in")
        nc.sync.dma_start(out=q_in[:, :, :512], in_=qT_t[:, :, :512])
        biasp = singles.tile([128, len(BIAS_NAMES), DT], FP32, tag="biasp")
        nc.sync.dma_start(out=biasp, in_=bias_d[:, :, :])
        nc.sync.dma_start(out=wqk[:, DT:, :], in_=wqk_t[:, DT:, :])      # wk

        ones_col = singles.tile([128, 1], BF16, tag="ones_col")
        nc.gpsimd.memset(ones_col, 1.0)
        ones_row = singles.tile([1, 128], FP32, tag="ones_row")
        nc.gpsimd.memset(ones_row, 1.0)
        eps_t = singles.tile([1, 1], FP32, tag="eps")
        nc.gpsimd.memset(eps_t, EPS)

        late = {}

        def load_early2():  # behind wq/q0/wk on the ring, before k1
            nc.sync.dma_start(out=q_in[:, :, 512:], in_=qT_t[:, :, 512:])
            t = singles.tile([128, 512], FP32, tag="mask")
            nc.sync.dma_start(out=t, in_=mask_d[:, :])
            late["mask"] = t
            t = singles.tile([128, DT, 512], BF16, tag="wv")
            nc.sync.dma_start(out=t, in_=wv_t)
            late["wv"] = t
            if general:
                t = singles.tile([128, 128], FP32, tag="ident")
                nc.sync.dma_start(out=t, in_=ident_d[:, :])
                late["ident"] = t
                t = singles.tile([1, 512], BF16, tag="bvrow")
                nc.gpsimd.dma_start(
                    out=t, in_=bass.AP(tensor=bvrow_d, offset=0, ap=[[0, 1], [1, 512]])
                )
                late["bvrow"] = t

        def load_late():
            t = singles.tile([128, 2 * DT, 512], BF16, tag="w12")
            nc.sync.dma_start(out=t, in_=w12_t)
            late["w12"] = t

        def bias_ap(name, dt_):
            return biasp[:, BIX[name], dt_ : dt_ + 1]

        qTp = singles.tile([128, DT, WPC], BF16, tag="qTp")

        def proj(w_sb, w_off, bias_name, in_sb, in_off, n, out_sb_ap):
            """out = relu(x @ W + b), epilogue on ACT."""
            for do in range(DT):
                ps = ps_proj.tile([128, 512], FP32, tag="proj_ps", name="proj_ps")
                ps = ps[:, :n]
                for ki in range(DT):
                    nc.tensor.matmul(
                        ps, lhsT=w_sb[:, w_off + ki, ts(do, 128)],
                        rhs=in_sb[:, ki, ds(in_off, n)],
                        start=(ki == 0), stop=(ki == DT - 1),
                    )
                nc.scalar.activation(
                    out=out_sb_ap(do), in_=ps, func=AF.Relu,
                    bias=bias_ap(bias_name, do), scale=1.0,
                )

        # ---- q projection, first half (only wq + q0 loaded yet) ----
        proj(wqk, 0, "bq", q_in, 0, 512, lambda do: qTp[:, do, ds(0, 512)])

        # ---- attention blocks ----
        kv_tiles = [None, None]

        def emit_kv_load(j):  # loads blocks 2j, 2j+1; v on the gpsimd queue
            k_in = kin_p.tile([128, DT, 1024], BF16, tag="k_in", name="k_in")
            nc.sync.dma_start(out=k_in, in_=kT_t[:, :, ts(j, 1024)])
            v_in = vin_p.tile([128, 2, 2048], BF16, tag="v_in", name="v_in")
            nc.gpsimd.dma_start(out=v_in, in_=vN_t[j])
            kv_tiles[0], kv_tiles[1] = k_in, v_in

        def emit_block(b, aoT, load=True):
            t = b % 2
            if t == 0 and load:
                emit_kv_load(b // 2)
            k_in, v_in = kv_tiles

            kTp = ktp_p.tile([128, DT, 512], BF16, tag="kTp", name="kTp")
            proj(wqk, DT, "bk", k_in, t * 512, 512, lambda do: kTp[:, do, :])

            sc_ps = ps_sc.tile([128, 512], FP32, tag="sc_ps", name="sc_ps")
            for ki in range(DT):
                nc.tensor.matmul(
                    sc_ps, lhsT=qTp[:, ki, ts(b, 128)], rhs=kTp[:, ki, :],
                    start=(ki == 0), stop=(ki == DT - 1),
                )
            # band extraction: wts[p, f] = sc[p, 4p+f]
            sm = att_p.tile([128, 512], BF16, tag="sm", name="sm")
            nc.vector.tensor_tensor(sm, sc_ps, late["mask"], op=OP.mult)
            wts = small.tile([128, F], FP32, tag="wts", name="wts")
            nc.vector.tensor_reduce(
                out=wts, in_=sm.rearrange("p (kw f) -> p f kw", f=F),
                axis=mybir.AxisListType.X, op=OP.add,
            )
            # windowed downsample of raw value (bf16)
            ao = att_p.tile([128, 512], BF16, tag="ao", name="ao")
            vv = v_in[:, t, :]
            nc.vector.tensor_scalar(
                out=ao, in0=vv[:, ts(0, 512)], scalar1=wts[:, 0:1], scalar2=None,
                op0=OP.mult,
            )
            for f in range(1, F):
                nc.vector.scalar_tensor_tensor(
                    out=ao, in0=vv[:, ts(f, 512)], scalar=wts[:, f : f + 1],
                    in1=ao, op0=OP.mult, op1=OP.add,
                )
            if general:
                wsum = small.tile([128, 1], FP32, tag="wsum", name="wsum")
                nc.vector.tensor_reduce(
                    out=wsum, in_=wts, axis=mybir.AxisListType.X, op=OP.add
                )
                wsr_ps = ps_st.tile([1, 128], FP32, tag="wsr_ps", name="wsr_ps")
                nc.tensor.matmul(wsr_ps, lhsT=wsum, rhs=late["ident"],
                                 start=True, stop=True)
                nc.scalar.activation(
                    out=aoT["wsrow"][:, ts(aoT["bix"][b], 128)], in_=wsr_ps,
                    func=AF.Copy,
                )
            # move to transposed domain: aoT[p, o, w] = ao[w, 128o+p]
            nc.sync.dma_start_transpose(
                out=aoT["t"][:, :, ts(aoT["bix"][b], 128)], in_=ao
            )

        def emit_aot(blocks, width):
            t = aot_p.tile([128, DT, width], BF16, tag=f"aoT{width}", name="aoT")
            r = {"t": t, "bix": {b: i for i, b in enumerate(blocks)}}
            if general:
                r["wsrow"] = small.tile([1, width], BF16, tag=f"wsr{width}",
                                        name="wsrow")
            return r

        def emit_vproj_resid(col0, n, aoT):
            resid = sb_p.tile([128, DT, n], BF16, tag=f"resid{n}", name="resid")
            for do in range(DT):
                ps = ps_proj.tile([128, 512], FP32, tag="proj_ps", name="vproj_ps")
                ps = ps[:, :n]
                for ki in range(DT):
                    nc.tensor.matmul(
                        ps, lhsT=late["wv"][:, ki, ts(do, 128)], rhs=aoT["t"][:, ki, :],
                        start=(ki == 0),
                        stop=(ki == DT - 1 and not general),
                    )
                if general:
                    nc.tensor.matmul(
                        ps, lhsT=late["bvrow"][:, ts(do, 128)], rhs=aoT["wsrow"],
                        start=False, stop=True,
                    )
                nc.vector.tensor_tensor(
                    resid[:, do, :], ps, qTp[:, do, ds(col0, n)], op=OP.add
                )
            return resid

        def emit_ln(x_sb, n, out_cb):
            """LayerNorm over D of x_sb [128, DT, n] (bf16, transposed).
            out_cb(dt, y2) consumes normalized (pre-affine) tiles, which stay
            alive in the returned scratch tile."""
            scr = sb_p.tile([128, DT, n], BF16, tag=f"scr{n}", name="scr")
            nc.vector.tensor_tensor(
                scr.rearrange("p a b -> p (a b)"),
                x_sb.rearrange("p a b -> p (a b)"),
                x_sb.rearrange("p a b -> p (a b)"), op=OP.mult,
            )
            mean_ps = ps_st.tile([1, 512], FP32, tag="st_mean", name="st_mean")
            mean_ps = mean_ps[:, :n]
            for ki in range(DT):
                nc.tensor.matmul(
                    mean_ps, lhsT=ones_col, rhs=x_sb[:, ki, :],
                    start=(ki == 0), stop=(ki == DT - 1),
                )
            sq_ps = ps_st.tile([1, 512], FP32, tag="st_sq", name="st_sq")
            sq_ps = sq_ps[:, :n]
            for ki in range(DT):
                nc.tensor.matmul(
                    sq_ps, lhsT=ones_col, rhs=scr[:, ki, :],
                    start=(ki == 0), stop=(ki == DT - 1),
                )
            mean_sb = small.tile([1, 512], FP32, tag="mean_sb", name="mean_sb")
            mean_sb = mean_sb[:, :n]
            nc.scalar.activation(out=mean_sb, in_=mean_ps, func=AF.Copy, scale=1.0 / D)
            m2 = small.tile([1, 512], FP32, tag="m2", name="m2")
            m2 = m2[:, :n]
            nc.scalar.activation(out=m2, in_=mean_ps, func=AF.Square, scale=1.0 / D)
            var = small.tile([1, 512], FP32, tag="var", name="var")
            var = var[:, :n]
            nc.vector.scalar_tensor_tensor(
                out=var, in0=sq_ps, scalar=1.0 / D, in1=m2,
                op0=OP.mult, op1=OP.subtract,
            )
            nc.scalar.activation(out=var, in_=var, func=AF.Sqrt, bias=eps_t, scale=1.0)
            rstd_sb = small.tile([1, 512], FP32, tag="rstd_sb", name="rstd_sb")
            rstd_sb = rstd_sb[:, :n]
            nc.vector.reciprocal_approx_fast(out=rstd_sb, in_=var)

            bc = {}
            for nm, row in (("mu", mean_sb), ("rs", rstd_sb)):
                bps = ps_bc.tile([128, 512], FP32, tag="bc_ps", name="bc_" + nm)
                bps = bps[:, :n]
                nc.tensor.matmul(bps, lhsT=ones_row, rhs=row, start=True, stop=True)
                bsb = small.tile([128, 512], BF16, tag="bc_sb", name="bcs_" + nm)
                bsb = bsb[:, :n]
                nc.scalar.activation(out=bsb, in_=bps, func=AF.Copy)
                bc[nm] = bsb
            for dt_ in range(DT):
                y = scr[:, dt_, :]  # reuse squares tile as y2 output
                nc.vector.tensor_tensor(y, x_sb[:, dt_, :], bc["mu"], op=OP.subtract)
                nc.vector.tensor_tensor(y, y, bc["rs"], op=OP.mult)
                out_cb(dt_, y)
            return scr

        def emit_ln1(col0, n, resid):
            if general:
                xT = sb_p.tile([128, DT, n], BF16, tag=f"xT{n}", name="xT")

                def write_x(dt_, y):
                    nc.scalar.activation(
                        out=xT[:, dt_, :], in_=y, func=AF.Identity,
                        bias=bias_ap("gb1", dt_), scale=bias_ap("g1", dt_),
                    )
                y2 = emit_ln(resid, n, write_x)
                return {"y2": y2, "x": xT}
            y2 = emit_ln(resid, n, lambda dt_, y: None)
            return {"y2": y2, "x": y2}

        def emit_ffn(col0, n, st):
            hT = sb_p.tile([128, DT, n], BF16, tag=f"hT{n}", name="hT")
            proj(late["w12"], 0, "b1", st["x"], 0, n, lambda ht: hT[:, ht, :])

            resid2 = sb_p.tile([128, DT, n], BF16, tag=f"resid2_{n}", name="resid2")
            for do in range(DT):
                ps = ps_proj.tile([128, 512], FP32, tag="proj_ps", name="ffn2_ps")
                ps = ps[:, :n]
                for ki in range(DT):
                    nc.tensor.matmul(
                        ps, lhsT=late["w12"][:, DT + ki, ts(do, 128)],
                        rhs=hT[:, ki, :],
                        start=(ki == 0), stop=(ki == DT - 1),
                    )
                if general:
                    nc.vector.scalar_tensor_tensor(
                        out=resid2[:, do, :], in0=ps, scalar=bias_ap("b2", do),
                        in1=st["x"][:, do, :], op0=OP.add, op1=OP.add,
                    )
                else:
                    nc.vector.scalar_tensor_tensor(
                        out=resid2[:, do, :], in0=st["y2"][:, do, :],
                        scalar=bias_ap("g1", do), in1=ps, op0=OP.mult, op1=OP.add,
                    )
            return resid2

        def emit_ln2_out(col0, n, resid2, split_dma):
            out_sb = sb_p.tile([128, DT, n], BF16, tag=f"out{n}", name="out_sb")

            def write_out(dt_, y):
                nc.scalar.activation(
                    out=out_sb[:, dt_, :], in_=y, func=AF.Identity,
                    bias=bias_ap("gb2", dt_), scale=bias_ap("g2", dt_),
                )
                if split_dma:
                    nc.gpsimd.dma_start(
                        out=outT_t[:, dt_, ds(col0, n)], in_=out_sb[:, dt_, :]
                    )
            emit_ln(resid2, n, write_out)
            if not split_dma:
                nc.gpsimd.dma_start(out=outT_t[:, :, ds(col0, n)], in_=out_sb)

        # ---- main schedule ----
        c0, c1, c2 = CHUNKS
        aoT0 = emit_aot(c0[2], c0[1])
        emit_kv_load(0)
        load_early2()
        emit_block(0, aoT0, load=False)
        # q projection, second half (q1 now behind wk/k0 on the ring)
        proj(wqk, 0, "bq", q_in, 512, 512, lambda do: qTp[:, do, ds(512, 512)])
        emit_block(1, aoT0)
        load_late()
        emit_block(2, aoT0)
        emit_block(3, aoT0)
        r0 = emit_vproj_resid(c0[0], c0[1], aoT0)
        aoT1 = emit_aot(c1[2], c1[1])
        emit_block(4, aoT1)
        st0 = emit_ln1(c0[0], c0[1], r0)
        emit_block(5, aoT1)
        r1 = emit_vproj_resid(c1[0], c1[1], aoT1)
        aoT2 = emit_aot(c2[2], c2[1])
        emit_block(6, aoT2)
        rr0 = emit_ffn(c0[0], c0[1], st0)
        emit_block(7, aoT2)
        st1 = emit_ln1(c1[0], c1[1], r1)
        r2 = emit_vproj_resid(c2[0], c2[1], aoT2)
        emit_ln2_out(c0[0], c0[1], rr0, split_dma=False)
        rr1 = emit_ffn(c1[0], c1[1], st1)
        st2 = emit_ln1(c2[0], c2[1], r2)
        emit_ln2_out(c1[0], c1[1], rr1, split_dma=False)
        rr2 = emit_ffn(c2[0], c2[1], st2)
        emit_ln2_out(c2[0], c2[1], rr2, split_dma=True)

    nc.finalize()
    return nc


def _band_mask():
    p = np.arange(128)[:, None]
    k = np.arange(512)[None, :]
    band = (k - 4 * p >= 0) & (k - 4 * p <= 3)
    return band.astype(np.float32)


def _is_general(inputs):
    f32 = lambda n: np.asarray(inputs[n], dtype=np.float32)
    return bool(
        np.any(f32("b_v")) or np.any(f32("ln1_b")) or np.any(f32("ffn_b2"))
    )


def _host_prep(inputs, general):
    """Shared (per-core-invariant) tensors, host-side precompute."""
    import ml_dtypes

    BF = ml_dtypes.bfloat16
    f32 = lambda x: np.asarray(x, dtype=np.float32)

    def colpack(v):  # [D] -> [128, DT] column tile layout (d = o*128 + p)
        return f32(v).reshape(DT, 128).T

    wq, wk, wv = f32(inputs["w_q"]), f32(inputs["w_k"]), f32(inputs["w_v"])
    w1, w2 = f32(inputs["ffn_w1"]), f32(inputs["ffn_w2"])
    g1 = f32(inputs["ln1_g"])
    if not general:
        w1 = w1 * g1[:, None]  # fold LN1 gain into FFN1 (gb1 == 0)

    shared = {
        "wqk16": np.ascontiguousarray(np.concatenate([wq, wk], axis=0)).astype(BF),
        "w12_16": np.ascontiguousarray(np.concatenate([w1, w2], axis=0)).astype(BF),
        "wv16": np.ascontiguousarray(wv).astype(BF),
        "cident": np.eye(128, dtype=np.float32),
        "cmask": _band_mask(),
        "bvrow16": f32(inputs["b_v"]).astype(BF),
    }
    bias_cols = {
        "bq": f32(inputs["b_q"]), "bk": f32(inputs["b_k"]),
        "b1": f32(inputs["ffn_b1"]), "b2": f32(inputs["ffn_b2"]),
        "g1": g1, "gb1": f32(inputs["ln1_b"]),
        "g2": f32(inputs["ln2_g"]), "gb2": f32(inputs["ln2_b"]),
    }
    bp = np.stack([colpack(bias_cols[n]) for n in BIAS_NAMES], axis=1)
    shared["biaspack"] = np.ascontiguousarray(bp)  # [128, NB, DT]
    return shared


def kernel(**inputs):
    import ml_dtypes

    BF = ml_dtypes.bfloat16

    general = _is_general(inputs)
    key_ = ("prog", general)
    if key_ not in _CACHE:
        _CACHE[key_] = build_program(general)
    nc = _CACHE[key_]

    shared = _host_prep(inputs, general)
    query = np.asarray(inputs["query"], dtype=np.float32)
    key_t = np.asarray(inputs["key"], dtype=np.float32)
    value = np.asarray(inputs["value"], dtype=np.float32)

    in_maps = []
    for c in range(NCORES):
        bi, half = c // 2, c % 2
        w0 = half * WPC
        m = dict(shared)
        m["qT16"] = np.ascontiguousarray(query[bi, w0 : w0 + WPC, :].T).astype(BF)
        m["kT16"] = np.ascontiguousarray(
            key_t[bi, w0 * F : (w0 + WPC) * F, :].T).astype(BF)
        m["vN"] = np.ascontiguousarray(
            value[bi, w0 * F : (w0 + WPC) * F, :]).astype(BF)
        in_maps.append(m)

    res = run_bass_kernel_spmd(nc, in_maps, core_ids=list(range(NCORES)))
    _CACHE["last_result"] = res
    out = np.empty((B, SQ, D), dtype=np.float32)
    for c in range(NCORES):
        bi, half = c // 2, c % 2
        w0 = half * WPC
        out[bi, w0 : w0 + WPC, :] = res.results[c]["outT"].astype(np.float32).T
    return out
